# revision 13
# baseline (speedup 1.0000x reference)
"""GAT layer kernel for 8 Trainium2 NeuronCores.

Row-shards the N=8192 nodes across 8 cores (1024 rows each). Each core:
  - computes full Wh_aug = h @ [W | W@a1 | W@a2] (bf16 matmul, fp32 psum)
  - builds p[j, i] = exp(lrelu(f_src_i + f_dst_j, 0.2)) * adj[i, j]  in a
    j-on-partitions layout (adj tiles are PE-transposed through PSUM; the
    DVE mask-multiply drains PSUM)
  - aggregates UT[65, 1024] += Whaug[j_tile].T @ p  (ones column gives the
    softmax row-sums), then normalizes + ELU and writes its out rows.

softmax needs no max-subtraction here: z = f_src + f_dst is bounded (~|13|),
exp fits comfortably in fp32, and att = p / sum(p) is invariant to the
multiplicative 0/1 mask.
"""

import numpy as np

N = 8192
F_IN = 256
F_OUT = 64
M = 65  # F_OUT + ones column (rowsum)
NCORES = 8
ROWS = N // NCORES  # 1024 rows per core
NT = N // 128  # 64 j-tiles
IT = ROWS // 128  # 8 i-tiles per core
ALPHA = 0.2

_PROGRAM = None


def _build_program(debug=False):
    from contextlib import ExitStack

    import concourse.bacc as bacc
    import concourse.mybir as mybir
    import concourse.tile as tile
    from concourse.masks import make_identity

    f32 = mybir.dt.float32
    bf16 = mybir.dt.bfloat16
    i32 = mybir.dt.int32
    AF = mybir.ActivationFunctionType
    OP = mybir.AluOpType

    nc = bacc.Bacc(
        "TRN2",
        target_bir_lowering=False,
        debug=False,
        enable_asserts=False,
        num_devices=NCORES,
    )

    h = nc.dram_tensor("h", [N, F_IN], f32, kind="ExternalInput").ap()
    h_own = nc.dram_tensor("h_own", [ROWS, F_IN], f32, kind="ExternalInput").ap()
    adj_own = nc.dram_tensor("adj_own", [ROWS, N], i32, kind="ExternalInput").ap()
    W = nc.dram_tensor("W", [F_IN, F_OUT], f32, kind="ExternalInput").ap()
    a = nc.dram_tensor("a", [2 * F_OUT, 1], f32, kind="ExternalInput").ap()
    out = nc.dram_tensor("out", [ROWS, F_OUT], f32, kind="ExternalOutput").ap()
    if debug:
        dbg_fdst = nc.dram_tensor("dbg_fdst", [128, NT], f32, kind="ExternalOutput").ap()
        dbg_fsrcb = nc.dram_tensor(
            "dbg_fsrcb", [128, ROWS], f32, kind="ExternalOutput"
        ).ap()
        dbg_whaug = nc.dram_tensor(
            "dbg_whaug", [128, NT, M], f32, kind="ExternalOutput"
        ).ap()
        dbg_p0 = nc.dram_tensor("dbg_p0", [128, ROWS], f32, kind="ExternalOutput").ap()
        dbg_adjT0 = nc.dram_tensor(
            "dbg_adjT0", [128, ROWS], f32, kind="ExternalOutput"
        ).ap()
        dbg_zl0 = nc.dram_tensor("dbg_zl0", [128, ROWS], f32, kind="ExternalOutput").ap()
        dbg_uts = nc.dram_tensor("dbg_uts", [M, ROWS], f32, kind="ExternalOutput").ap()
        dbg_fin0 = nc.dram_tensor("dbg_fin0", [128, M], f32, kind="ExternalOutput").ap()
        dbg_hn0 = nc.dram_tensor("dbg_hn0", [128, F_OUT], f32, kind="ExternalOutput").ap()

    with tile.TileContext(nc) as tc, ExitStack() as ctx:
        # ---- persistent pools -------------------------------------------------
        pers = ctx.enter_context(tc.tile_pool(name="pers", bufs=1))
        psum_ut = ctx.enter_context(tc.tile_pool(name="psum_ut", bufs=1, space="PSUM"))

        ident = pers.tile([128, 128], f32, tag="ident")
        make_identity(nc, ident)

        # Whaug: per j-tile [128, 65] bf16 lhsT (64 cols of Wh + ones col)
        whaug = pers.tile([128, NT, M], bf16, tag="whaug")
        fdst = pers.tile([128, NT], f32, tag="fdst")  # col t = f_dst[t*128:(t+1)*128]
        fsrc_bcast = pers.tile([128, ROWS], f32, tag="fsrc_bcast")

        # UT accumulator: [65, 1024] as two single-bank halves
        ut_lo = psum_ut.tile([M, 512], f32, tag="ut_lo")
        ut_hi = psum_ut.tile([M, 512], f32, tag="ut_hi")

        # ---- setup: W_aug = [W | W@a1 | W@a2]  (bf16 [128, 2, 66]) -----------
        with (
            tc.tile_pool(name="setup", bufs=1) as setup,
            tc.tile_pool(name="psum_setup", bufs=1, space="PSUM") as psum_setup,
        ):
            w_sb = setup.tile([128, 2, F_OUT], f32, tag="w_sb")
            nc.sync.dma_start(w_sb[:], W.rearrange("(c p) f -> p c f", p=128))
            a_resh = setup.tile([F_OUT, 2], f32, tag="a_resh")
            nc.sync.dma_start(a_resh[:], a.rearrange("(c f) one -> f (c one)", c=2))

            wt_ps = psum_setup.tile([F_OUT, 256], f32, tag="wt_ps")
            for k in range(2):
                nc.tensor.transpose(wt_ps[:, k * 128 : (k + 1) * 128], w_sb[:, k], ident)
            wt_sb = setup.tile([F_OUT, 256], f32, tag="wt_sb")
            nc.vector.tensor_copy(wt_sb[:], wt_ps[:])

            waug_w = setup.tile([128, 2, 66], bf16, tag="waug_w")
            wa_ps = psum_setup.tile([128, 2], f32, tag="wa_ps")
            for k in range(2):
                nc.tensor.matmul(
                    wa_ps[:],
                    wt_sb[:, k * 128 : (k + 1) * 128],
                    a_resh[:],
                    start=True,
                    stop=True,
                )
                nc.vector.tensor_copy(waug_w[:, k, :F_OUT], w_sb[:, k])
                nc.vector.tensor_copy(waug_w[:, k, F_OUT:], wa_ps[:])

            # ---- phase A: Wh_aug + f_dst (all nodes), f_src (own rows) -------
            with (
                tc.tile_pool(name="pha", bufs=2) as pha,
                tc.tile_pool(name="pha_big", bufs=1) as pha_big,
                tc.tile_pool(name="psum_a", bufs=2, space="PSUM") as psum_a,
            ):
                h_bf = pha_big.tile([128, NT, F_IN], bf16, tag="h_bf")
                nc.gpsimd.dma_start(h_bf[:], h.rearrange("(n p) f -> p n f", p=128))
                ho_bf = pha_big.tile([128, IT, F_IN], bf16, tag="ho_bf")
                nc.gpsimd.dma_start(ho_bf[:], h_own.rearrange("(n p) f -> p n f", p=128))

                for nt in range(NT):
                    hT = pha.tile([128, F_IN], bf16, tag="hT")
                    for k in range(2):
                        nc.sync.dma_start_transpose(
                            hT[:, k * 128 : (k + 1) * 128],
                            h_bf[:, nt, k * 128 : (k + 1) * 128],
                        )
                    wh_ps = psum_a.tile([128, 66], f32, tag="wh_ps")
                    for k in range(2):
                        nc.tensor.matmul(
                            wh_ps[:],
                            hT[:, k * 128 : (k + 1) * 128],
                            waug_w[:, k, :],
                            start=(k == 0),
                            stop=(k == 1),
                        )
                    nc.vector.tensor_copy(whaug[:, nt, :F_OUT], wh_ps[:, :F_OUT])
                    nc.vector.tensor_copy(fdst[:, nt : nt + 1], wh_ps[:, 65:66])

                nc.vector.memset(whaug[:, :, F_OUT], 1.0)

                fsrc8 = pha_big.tile([128, IT], f32, tag="fsrc8")
                for ot in range(IT):
                    hT = pha.tile([128, F_IN], bf16, tag="hT")
                    for k in range(2):
                        nc.sync.dma_start_transpose(
                            hT[:, k * 128 : (k + 1) * 128],
                            ho_bf[:, ot, k * 128 : (k + 1) * 128],
                        )
                    wh_ps = psum_a.tile([128, 66], f32, tag="wh_ps")
                    for k in range(2):
                        nc.tensor.matmul(
                            wh_ps[:],
                            hT[:, k * 128 : (k + 1) * 128],
                            waug_w[:, k, :],
                            start=(k == 0),
                            stop=(k == 1),
                        )
                    nc.vector.tensor_copy(fsrc8[:, ot : ot + 1], wh_ps[:, 64:65])

                # f_src broadcast: [128, 8] -> (T) -> [8, 128] -> [1, 1024] -> all partitions
                f8_ps = psum_a.tile([IT, 128], f32, tag="f8_ps")
                nc.tensor.transpose(f8_ps[:], fsrc8[:], ident)
                f8t = pha.tile([IT, 128], f32, tag="f8t")
                nc.vector.tensor_copy(f8t[:], f8_ps[:])
                frow = pha_big.tile([1, ROWS], f32, tag="frow")
                for s in range(IT):
                    nc.sync.dma_start(
                        frow[0:1, s * 128 : (s + 1) * 128], f8t[s : s + 1, :]
                    )
                nc.gpsimd.partition_broadcast(fsrc_bcast[:], frow[:])

        # ---- phase B: main streaming loop over j-tiles ------------------------
        with (
            tc.tile_pool(name="phb", bufs=3) as phb,
            tc.tile_pool(name="phb2", bufs=3) as phb2,
            tc.tile_pool(name="psum_b", bufs=2, space="PSUM") as psum_b,
        ):
            for t in range(NT):
                adj_sb = phb.tile([128, IT, 128], f32, tag="adj_sb")
                nc.gpsimd.dma_start(
                    adj_sb[:],
                    adj_own[:, t * 128 : (t + 1) * 128].rearrange(
                        "(s p) j -> p s j", p=128
                    ),
                )
                adjT = psum_b.tile([128, ROWS], f32, tag="adjT")
                for s in range(IT):
                    nc.tensor.transpose(
                        adjT[:, s * 128 : (s + 1) * 128],
                        adj_sb[:, s, :],
                        ident,
                    )
                zl = phb.tile([128, ROWS], f32, tag="zl")
                nc.scalar.activation(
                    zl[:],
                    fsrc_bcast[:],
                    AF.Prelu,
                    bias=fdst[:, t : t + 1],
                    scale=1.0,
                    alpha=ALPHA,
                )
                q = phb2.tile([128, ROWS], bf16, tag="q")
                nc.scalar.activation(q[:], zl[:], AF.Exp)
                p = phb2.tile([128, ROWS], bf16, tag="p")
                nc.vector.tensor_tensor(p[:], q[:], adjT[:], OP.mult)
                if debug and t == 0:
                    dstage = phb2.tile([128, ROWS], f32, tag="dstage")
                    nc.vector.tensor_copy(dstage[:], p[:])
                    nc.sync.dma_start(dbg_p0[:], dstage[:])
                    dstage2 = phb2.tile([128, ROWS], f32, tag="dstage2")
                    nc.vector.tensor_copy(dstage2[:], adjT[:])
                    nc.sync.dma_start(dbg_adjT0[:], dstage2[:])
                    nc.sync.dma_start(dbg_zl0[:], zl[:])
                nc.tensor.matmul(
                    ut_lo[:],
                    whaug[:, t, :],
                    p[:, :512],
                    start=(t == 0),
                    stop=(t == NT - 1),
                )
                nc.tensor.matmul(
                    ut_hi[:],
                    whaug[:, t, :],
                    p[:, 512:],
                    start=(t == 0),
                    stop=(t == NT - 1),
                )

        # ---- phase C: normalize + ELU + store --------------------------------
        with (
            tc.tile_pool(name="phc", bufs=2) as phc,
            tc.tile_pool(name="psum_c", bufs=2, space="PSUM") as psum_c,
        ):
            uts = phc.tile([M, ROWS], f32, tag="uts")
            nc.vector.tensor_copy(uts[:, :512], ut_lo[:])
            nc.vector.tensor_copy(uts[:, 512:], ut_hi[:])
            if debug:
                nc.sync.dma_start(dbg_uts[:], uts[:])
                nc.sync.dma_start(dbg_fdst[:], fdst[:])
                nc.sync.dma_start(dbg_fsrcb[:], fsrc_bcast[:])
                dw = phc.tile([128, NT, M], f32, tag="dw")
                nc.vector.tensor_copy(dw[:], whaug[:])
                nc.sync.dma_start(dbg_whaug[:], dw[:])
            for it in range(IT):
                fin_ps = psum_c.tile([128, M], f32, tag="fin_ps")
                nc.tensor.transpose(
                    fin_ps[:], uts[:, it * 128 : (it + 1) * 128], ident[:M, :M]
                )
                rs = phc.tile([128, 1], f32, tag="rs")
                nc.vector.reciprocal(rs[:], fin_ps[:, 64:65])
                hn = phc.tile([128, F_OUT], f32, tag="hn")
                nc.vector.tensor_scalar(hn[:], fin_ps[:, :F_OUT], rs[:], None, OP.mult)
                if debug and it == 0:
                    dfin = phc.tile([128, M], f32, tag="dfin")
                    nc.vector.tensor_copy(dfin[:], fin_ps[:])
                    nc.sync.dma_start(dbg_fin0[:], dfin[:])
                    nc.sync.dma_start(dbg_hn0[:], hn[:])
                em = phc.tile([128, F_OUT], f32, tag="em")
                nc.vector.tensor_scalar(em[:], hn[:], 0.0, None, OP.min)
                nc.scalar.activation(em[:], em[:], AF.Exp)
                o = phc.tile([128, F_OUT], f32, tag="o")
                # (hn max 0) + (exp(min(hn,0)) - 1)  == ELU(hn)
                nc.vector.tensor_scalar(em[:], em[:], 1.0, None, OP.subtract)
                nc.vector.scalar_tensor_tensor(o[:], hn[:], 0.0, em[:], OP.max, OP.add)
                nc.sync.dma_start(out[it * 128 : (it + 1) * 128, :], o[:])

    nc.compile()
    return nc


def _get_program():
    global _PROGRAM
    if _PROGRAM is None:
        _PROGRAM = _build_program()
    return _PROGRAM


def kernel(h, adj, W, a):
    from concourse.bass_utils import run_bass_kernel_spmd

    h = np.ascontiguousarray(np.asarray(h, dtype=np.float32))
    adj = np.ascontiguousarray(np.asarray(adj, dtype=np.int32))
    W = np.ascontiguousarray(np.asarray(W, dtype=np.float32))
    a = np.ascontiguousarray(np.asarray(a, dtype=np.float32))

    nc = _get_program()
    in_maps = []
    for c in range(NCORES):
        in_maps.append(
            {
                "h": h,
                "h_own": np.ascontiguousarray(h[c * ROWS : (c + 1) * ROWS]),
                "adj_own": np.ascontiguousarray(adj[c * ROWS : (c + 1) * ROWS]),
                "W": W,
                "a": a,
            }
        )
    res = run_bass_kernel_spmd(nc, in_maps, core_ids=list(range(NCORES)))
    return np.concatenate([res.results[c]["out"] for c in range(NCORES)], axis=0)
